# revision 14
# baseline (speedup 1.0000x reference)
"""GPT-2 (12L, 768C, 12H, T=1024, B=2) forward pass on 8 Trainium2 NeuronCores.

Sharding: 2x4 grid. Core c -> batch b=c//4, tensor-parallel shard g=c%4.
Within a batch group of 4 cores (Megatron-style):
  - attention: 3 heads per core (qkv column-sharded), output proj row-sharded,
    partial sums all-reduced (bf16) across the group.
  - MLP: fc column-sharded (768 cols/core), mproj row-sharded, all-reduced.
  - lm_head: vocab-sharded; host reassembles [B, T, V].
Residual stream x stays fp32 in SBUF in [t, c] layout; all matmuls run in
bf16 with fp32 PSUM accumulation. LayerNorm scale/bias are folded into the
following matmul's weights on the host; the device applies only
(x - mean) * rsqrt(var + eps) with torch-style unbiased variance.
rsqrt runs on the vector engine (magic-constant seed + 2 Newton steps) so the
scalar engine only ever needs {Copy, Exp, Square, Tanh} - one activation
table set, zero mid-kernel table reloads.
Attention uses a transposed-score layout (scores^T [s, t]); the softmax
denominator comes free as a 65th output row of the att@V matmul (V carries a
ones column); causal structure skips the strictly-lower-triangular compute
and masks the diagonal block.
Elementwise work is spread across engines: DVE (LN stats/normalize, softmax
divide, gelu polynomial), Activation (PSUM->SBUF copies, exp, tanh, square),
Pool (residual adds, V copies, gelu PSUM reads, collectives).
"""

import sys

sys.path.insert(0, "/opt/trn_rl_repo")

import numpy as np
import ml_dtypes

import concourse.bass as bass
import concourse.mybir as mybir
import concourse.tile as tile
from concourse import bacc
from concourse.bass_utils import run_bass_kernel_spmd
from concourse.masks import make_identity

L, H, C, V, BLK = 12, 12, 768, 50257, 1024
B, T, D = 2, 1024, 64
TP = 4                      # tensor-parallel group size
HL = H // TP                # heads per core (3)
HC = HL * D                 # head cols per core (192)
FS = 4 * C // TP            # fc cols per core (768)
NKC = C // 128              # k-tiles over C (6)
NTT = T // 128              # t-tiles (8)
import os as _os_env
NM_HD = int(_os_env.environ.get("GPT2_NMHD", "99"))  # lm-head m-tiles (99*128 = 12672 >= ceil(V/4))
VPAD = NM_HD * 128
CG = float(np.sqrt(2.0 / np.pi))
MAGIC = 0x5F3759DF
GROUPS = [[0, 1, 2, 3], [4, 5, 6, 7]]

f32 = mybir.dt.float32
bf16 = mybir.dt.bfloat16
i32 = mybir.dt.int32
ALU = mybir.AluOpType
ACTF = mybir.ActivationFunctionType
COPY = ACTF.Copy

_CACHE = {}


def _bf(a):
    return np.ascontiguousarray(a.astype(ml_dtypes.bfloat16))


def _f32(a):
    return np.ascontiguousarray(a.astype(np.float32))


def _build(U, nl, v_bias_nz, ap_bias_nz, mp_bias_nz):
    nc = bacc.Bacc("TRN2", target_bir_lowering=False, debug=False, num_devices=8)

    # ---------------- DRAM declarations ----------------
    idx_d = nc.dram_tensor("idx_t", [128, NTT], i32, kind="ExternalInput")
    wte_d = nc.dram_tensor("wte_c", [U, C], f32, kind="ExternalInput")
    wpe_d = nc.dram_tensor("wpe_t", [T, C], f32, kind="ExternalInput")
    wqk_d = nc.dram_tensor("wqk", [nl, 128, 3 * NKC * 128], bf16, kind="ExternalInput")
    wv_d = nc.dram_tensor("wv", [nl, 128, NKC * HC], bf16, kind="ExternalInput")
    wap_d = nc.dram_tensor("wap", [nl, 64, 3 * C], bf16, kind="ExternalInput")
    wfc_d = nc.dram_tensor("wfc", [nl, 128, NKC * NKC * 128], bf16, kind="ExternalInput")
    wmp_d = nc.dram_tensor("wmp", [nl, 128, NKC * C], bf16, kind="ExternalInput")
    whd_d = nc.dram_tensor("whd", [NM_HD, 128, NKC * 128], bf16, kind="ExternalInput")
    bqk_d = nc.dram_tensor("bqk", [128, nl * 4], f32, kind="ExternalInput")
    bfc_d = nc.dram_tensor("bfc", [128, nl * NKC], f32, kind="ExternalInput")
    bhd_d = nc.dram_tensor("bhd", [128, NM_HD], f32, kind="ExternalInput")
    bv_d = nc.dram_tensor("bv", [1, nl * HC], bf16, kind="ExternalInput")
    bap_d = nc.dram_tensor("bap", [1, nl * C], bf16, kind="ExternalInput")
    bmp_d = nc.dram_tensor("bmp", [1, nl * C], bf16, kind="ExternalInput")
    logits_d = nc.dram_tensor("logitsT", [VPAD, T], bf16, kind="ExternalOutput")

    ar_a_in = [nc.dram_tensor(f"ar_a_in{l}", [T, C], bf16) for l in range(nl)]
    ar_a_out = [nc.dram_tensor(f"ar_a_out{l}", [T, C], bf16) for l in range(nl)]
    ar_m_in = [nc.dram_tensor(f"ar_m_in{l}", [T, C], bf16) for l in range(nl)]
    ar_m_out = [nc.dram_tensor(f"ar_m_out{l}", [T, C], bf16) for l in range(nl)]

    with tile.TileContext(nc) as tc:
        with (
            tc.tile_pool(name="const", bufs=1) as cpool,
            tc.tile_pool(name="xres", bufs=1) as xpool,
            tc.tile_pool(name="work", bufs=2) as wk,
            tc.tile_pool(name="hT", bufs=13) as hTp,
            tc.tile_pool(name="qkp", bufs=4) as qkp,
            tc.tile_pool(name="vtp", bufs=9) as vtp,
            tc.tile_pool(name="oTp", bufs=4) as oTp,
            tc.tile_pool(name="expT", bufs=9) as expp,
            tc.tile_pool(name="wload", bufs=2) as wl,
            tc.tile_pool(name="whload", bufs=3) as whl,
            tc.tile_pool(name="psA", bufs=3, space="PSUM") as psA,
            tc.tile_pool(name="psT", bufs=2, space="PSUM") as psT,
            tc.tile_pool(name="psS", bufs=2, space="PSUM") as psS,
        ):
            # ---------------- constants ----------------
            ident = cpool.tile([128, 128], bf16)
            make_identity(nc, ident[:])
            tril = cpool.tile([128, 128], bf16)   # keep s<=t  (p<=f)
            nc.gpsimd.memset(tril[:], 1.0)
            nc.gpsimd.affine_select(
                out=tril[:], in_=tril[:], compare_op=ALU.is_ge, fill=0.0,
                base=0, pattern=[[1, 128]], channel_multiplier=-1)
            ones_s = cpool.tile([128, 64], bf16)
            nc.gpsimd.memset(ones_s[:], 1.0)
            ones_r = cpool.tile([1, 128], bf16)
            nc.gpsimd.memset(ones_r[:], 1.0)
            bqk_sb = cpool.tile([128, nl * 4], f32)
            nc.sync.dma_start(bqk_sb[:], bqk_d[:])
            bfc_sb = cpool.tile([128, nl * NKC], f32)
            nc.sync.dma_start(bfc_sb[:], bfc_d[:])
            bhd_sb = cpool.tile([128, NM_HD], f32)
            nc.sync.dma_start(bhd_sb[:], bhd_d[:])
            bv_sb = bap_sb = bmp_sb = None
            if v_bias_nz:
                bv_sb = cpool.tile([1, nl * HC], bf16)
                nc.sync.dma_start(bv_sb[:], bv_d[:])
            if ap_bias_nz:
                bap_sb = cpool.tile([1, nl * C], bf16)
                nc.sync.dma_start(bap_sb[:], bap_d[:])
            if mp_bias_nz:
                bmp_sb = cpool.tile([1, nl * C], bf16)
                nc.sync.dma_start(bmp_sb[:], bmp_d[:])

            # ---------------- embedding ----------------
            idx_sb = cpool.tile([128, NTT], i32)
            nc.sync.dma_start(idx_sb[:], idx_d[:])
            x = []
            for j in range(NTT):
                g_t = wk.tile([128, C], f32, tag="emb")
                nc.gpsimd.indirect_dma_start(
                    out=g_t[:], out_offset=None, in_=wte_d[:],
                    in_offset=bass.IndirectOffsetOnAxis(ap=idx_sb[:, j:j + 1], axis=0))
                p_t = wk.tile([128, C], f32, tag="pos")
                nc.sync.dma_start(p_t[:], wpe_d[128 * j:128 * (j + 1), :])
                xt = xpool.tile([128, C], f32, tag=f"x{j}")
                nc.gpsimd.tensor_tensor(out=xt[:], in0=g_t[:], in1=p_t[:], op=ALU.add)
                x.append(xt)

            # ---------------- helpers ----------------
            def layernorm_T(tag):
                """LN(x) (no affine; folded into weights), bf16, transposed.
                Returns 6 tiles [128, T] = h^T (c on partitions).
                rsqrt on DVE (magic seed + 2 Newton steps): the scalar engine
                never sees Sqrt, so its activation table is never reloaded."""
                hT = [[hTp.tile([128, 512], bf16, tag="hT", name=f"hT{_k}_{_n}")
                       for _n in range(2)] for _k in range(NKC)]
                stats = wk.tile([128, NTT, 2], f32, tag="lnstats")
                for j in range(NTT):
                    st6 = wk.tile([128, 2, 6], f32, tag="st6")
                    nc.vector.bn_stats(st6[:, 0, :], x[j][:, 0:384])
                    nc.vector.bn_stats(st6[:, 1, :], x[j][:, 384:768])
                    nc.vector.bn_aggr(stats[:, j, :], st6[:])
                # istd = rsqrt(var * C/(C-1) + eps) for all 8 tiles at once
                vc = wk.tile([128, NTT], f32, tag="ln_vc")
                nc.vector.tensor_scalar(
                    out=vc[:], in0=stats[:, :, 1], scalar1=float(C / (C - 1)),
                    scalar2=1e-5, op0=ALU.mult, op1=ALU.add)
                ti = wk.tile([128, NTT], i32, tag="ln_ti")
                nc.vector.tensor_scalar(
                    out=ti[:], in0=vc[:].bitcast(i32), scalar1=1, scalar2=None,
                    op0=ALU.arith_shift_right)
                nc.vector.tensor_scalar(
                    out=ti[:], in0=ti[:], scalar1=-1, scalar2=None,
                    op0=ALU.bitwise_xor)
                yi = wk.tile([128, NTT], i32, tag="ln_yi")
                nc.vector.tensor_scalar(
                    out=yi[:], in0=ti[:], scalar1=MAGIC + 1, scalar2=None,
                    op0=ALU.add)
                y = yi[:].bitcast(f32)
                yy = wk.tile([128, NTT], f32, tag="ln_yy")
                for _ in range(2):  # Newton: y *= 1.5 - 0.5*v*y*y
                    nc.vector.tensor_tensor(out=yy[:], in0=y, in1=y, op=ALU.mult)
                    nc.vector.tensor_tensor(out=yy[:], in0=yy[:], in1=vc[:], op=ALU.mult)
                    nc.vector.tensor_scalar(
                        out=yy[:], in0=yy[:], scalar1=-0.5, scalar2=1.5,
                        op0=ALU.mult, op1=ALU.add)
                    nc.vector.tensor_tensor(out=y, in0=y, in1=yy[:], op=ALU.mult)
                nmean = wk.tile([128, NTT], f32, tag="ln_nm")
                nc.vector.tensor_scalar_mul(nmean[:], stats[:, :, 0], -1.0)
                for j in range(NTT):
                    hb = wk.tile([128, C], bf16, tag="hbf")
                    nc.vector.tensor_scalar(
                        out=hb[:], in0=x[j][:], scalar1=nmean[:, j:j + 1],
                        scalar2=y[:, j:j + 1], op0=ALU.add, op1=ALU.mult)
                    for k in range(NKC):
                        ps = psT.tile([128, 128], bf16, tag="tp")
                        nc.tensor.transpose(ps[:], hb[:, 128 * k:128 * (k + 1)], ident[:])
                        nc.scalar.activation(
                            hT[k][j // 4][:, 128 * (j % 4):128 * (j % 4 + 1)],
                            ps[:], COPY)
                return hT

            import os as _os
            no_ar = _os.environ.get("GPT2_NO_AR", "0") == "1"
            ar_chunks = int(_os.environ.get("GPT2_AR_CHUNKS", "2"))
            mper = NTT // ar_chunks

            def ar_launch(xd_tiles, cc_in, ch):
                """DMA chunk ch of the partial delta out and start its AR."""
                if no_ar:
                    return
                for m in range(ch * mper, (ch + 1) * mper):
                    nc.sync.dma_start(cc_in[128 * m:128 * (m + 1), :], xd_tiles[m][:])

            def ar_collect(cc_in, cc_out, ch):
                if no_ar:
                    return
                r0, r1 = 128 * ch * mper, 128 * (ch + 1) * mper
                nc.gpsimd.collective_compute(
                    "AllReduce", ALU.add, replica_groups=GROUPS,
                    ins=[cc_in[r0:r1, :].opt()], outs=[cc_out[r0:r1, :].opt()])

            def ar_land(xd_tiles, cc_out, ch):
                """DMA chunk ch of the reduced delta back and add into x."""
                for m in range(ch * mper, (ch + 1) * mper):
                    if no_ar:
                        nc.gpsimd.tensor_tensor(
                            out=x[m][:], in0=x[m][:], in1=xd_tiles[m][:], op=ALU.add)
                        continue
                    ard = wk.tile([128, C], bf16, tag="ard")
                    nc.gpsimd.dma_start(ard[:], cc_out[128 * m:128 * (m + 1), :])
                    nc.gpsimd.tensor_tensor(
                        out=x[m][:], in0=x[m][:], in1=ard[:], op=ALU.add)

            # ---------------- transformer layers ----------------
            for l in range(nl):
                # layer weight loads (double-buffered; overlap with compute)
                wqk_sb = wl.tile([128, 3 * NKC * 128], bf16, tag="wqk")
                nc.sync.dma_start(wqk_sb[:], wqk_d[l])
                wv_sb = wl.tile([128, NKC * HC], bf16, tag="wv")
                nc.sync.dma_start(wv_sb[:], wv_d[l])
                wap_sb = wl.tile([64, 3 * C], bf16, tag="wap")
                nc.sync.dma_start(wap_sb[:], wap_d[l])
                wfc_sb = wl.tile([128, NKC * NKC * 128], bf16, tag="wfc", bufs=1)
                nc.sync.dma_start(wfc_sb[:], wfc_d[l])
                wmp_sb = wl.tile([128, NKC * C], bf16, tag="wmp", bufs=1)
                nc.sync.dma_start(wmp_sb[:], wmp_d[l])

                # ---- ln1 + qkv ----
                hT = layernorm_T(f"ln1_{l}")
                qk_meta = ((128, 0), (128, 768), (64, 1536), (64, 1920))
                qkT = [qkp.tile([mw, T], bf16, tag="qkT", name=f"qk{m}")
                       for m, (mw, moff) in enumerate(qk_meta)]
                for n in range(2):
                    for m, (mw, moff) in enumerate(qk_meta):
                        ps = psA.tile([mw, 512], f32, tag="big")
                        for k in range(NKC):
                            nc.tensor.matmul(
                                ps[:], wqk_sb[:, moff + k * mw:moff + (k + 1) * mw],
                                hT[k][n][:],
                                start=(k == 0), stop=(k == NKC - 1))
                        nc.scalar.activation(
                            qkT[m][:, 512 * n:512 * (n + 1)], ps[:], ACTF.Identity,
                            bias=bqk_sb[:mw, l * 4 + m:l * 4 + m + 1])

                vt = []
                for m in range(NTT):
                    vm = vtp.tile([128, HL, 65], bf16, tag="vt")
                    ps = psA.tile([128, HC], f32, tag="big")
                    nk = NKC + (1 if v_bias_nz else 0)
                    for k in range(NKC):
                        nc.tensor.matmul(
                            ps[:], hT[k][m // 4][:, 128 * (m % 4):128 * (m % 4 + 1)],
                            wv_sb[:, k * HC:(k + 1) * HC],
                            start=(k == 0), stop=(k == nk - 1))
                    if v_bias_nz:
                        nc.tensor.matmul(
                            ps[:], ones_r[:1, :128],
                            bv_sb[:1, l * HC:(l + 1) * HC],
                            start=False, stop=True)
                    for hh in range(HL):
                        nc.vector.tensor_copy(
                            vm[:, hh, 0:64], ps[:, 64 * hh:64 * (hh + 1)])
                    nc.gpsimd.memset(vm[:, :, 64:65], 1.0)
                    vt.append(vm)

                # ---- attention (scores^T layout, causal skip, free denom) ----
                oT = [oTp.tile([64, T], bf16, tag="oT", name=f"oT{_h}")
                      for _h in range(HL)]
                # ex[h][i]: exp'd scores^T for s-block i; only t >= 128*i kept
                ex = [[expp.tile([128, T - 128 * _i], bf16, tag=f"expT{_i}",
                                 name=f"ex{_h}_{_i}", bufs=3)
                       for _i in range(NTT)] for _h in range(HL)]
                xda = []
                for cch in range(2):
                    nblk = min(NTT, 4 * (cch + 1))
                    for hh in range(HL):
                        if hh < 2:
                            qT = qkT[0][64 * hh:64 * hh + 64, :]
                            kT = qkT[1][64 * hh:64 * hh + 64, :]
                        else:
                            qT = qkT[2][0:64, :]
                            kT = qkT[3][0:64, :]
                        for i in range(nblk):
                            toff = 128 * i
                            lo = max(toff, 512 * cch)
                            hi = 512 * (cch + 1)
                            if lo >= hi:
                                continue
                            ps = psA.tile([128, hi - lo], f32, tag="big")
                            nc.tensor.matmul(
                                ps[:], kT[:, toff:toff + 128],
                                qT[:, lo:hi], start=True, stop=True)
                            nc.scalar.activation(
                                ex[hh][i][:, lo - toff:hi - toff], ps[:], ACTF.Exp,
                                scale=float(D ** -0.5))
                            if lo == toff:  # diagonal block: mask s>t
                                nc.vector.tensor_tensor(
                                    out=ex[hh][i][:, 0:128],
                                    in0=ex[hh][i][:, 0:128], in1=tril[:],
                                    op=ALU.mult)
                        po = psS.tile([65, 512], f32, tag="po")
                        for i in range(nblk):
                            toff = 128 * i
                            lo = max(toff - 512 * cch, 0)
                            nc.tensor.matmul(
                                po[:, lo:512], vt[i][:, hh, :],
                                ex[hh][i][:, 512 * cch + lo - toff:
                                          512 * (cch + 1) - toff],
                                start=(i == 0), stop=(i == nblk - 1))
                        den = wk.tile([128, 512], bf16, tag="den")
                        nc.scalar.activation(den[64:65, :], po[64:65, :], COPY)
                        pb = psS.tile([64, 512], f32, tag="pb", bufs=1)
                        nc.tensor.matmul(
                            pb[:], ones_s[64:65, :], den[64:65, :],
                            start=True, stop=True)
                        rb = wk.tile([64, 512], f32, tag="rb")
                        nc.vector.reciprocal(rb[:], pb[:])
                        nc.vector.tensor_tensor(
                            out=oT[hh][:, 512 * cch:512 * (cch + 1)],
                            in0=po[0:64, :], in1=rb[:], op=ALU.mult)

                    # ---- attention output projection for this T-half ----
                    for m in range(4 * cch, 4 * (cch + 1)):
                        xm = wk.tile([128, C], bf16, tag="xd", bufs=8)
                        for n, (nlo, nw) in enumerate(((0, 512), (512, 256))):
                            ps = psA.tile([128, nw], f32, tag="big")
                            for kk in range(HL):
                                nc.tensor.matmul(
                                    ps[:], oT[kk][:, 128 * m:128 * (m + 1)],
                                    wap_sb[:, kk * C + nlo:kk * C + nlo + nw],
                                    start=(kk == 0),
                                    stop=(kk == HL - 1 and not ap_bias_nz))
                            if ap_bias_nz:
                                nc.tensor.matmul(
                                    ps[:], ones_r[:1, :128],
                                    bap_sb[:1, l * C + nlo:l * C + nlo + nw],
                                    start=False, stop=True)
                            nc.scalar.activation(xm[:, nlo:nlo + nw], ps[:], COPY)
                        xda.append(xm)
                    ch0 = 4 * cch // mper
                    for ch in range(ch0, (4 * (cch + 1)) // mper):
                        ar_launch(xda, ar_a_in[l], ch)
                        ar_collect(ar_a_in[l], ar_a_out[l], ch)
                for ch in range(ar_chunks):
                    ar_land(xda, ar_a_out[l], ch)

                # ---- ln2 + MLP ----
                h2T = layernorm_T(f"ln2_{l}")
                gT = [[hTp.tile([128, 512], bf16, tag="gT", name=f"gT{_k}_{_n}")
                      for _n in range(2)] for _k in range(NKC)]
                for n in range(2):
                    for m in range(NKC):
                        ps = psA.tile([128, 512], f32, tag="big")
                        for k in range(NKC):
                            nc.tensor.matmul(
                                ps[:], wfc_sb[:, (m * NKC + k) * 128:(m * NKC + k + 1) * 128],
                                h2T[k][n][:],
                                start=(k == 0), stop=(k == NKC - 1))
                        # u = ps + b ; gelu*2 = u * (1 + tanh(cg*(u + 0.044715 u^3)))
                        u = wk.tile([128, 512], f32, tag="gelu_u")
                        nc.vector.tensor_scalar_add(
                            u[:], ps[:], bfc_sb[:, l * NKC + m:l * NKC + m + 1])
                        t0 = wk.tile([128, 512], f32, tag="gelu_t0")
                        nc.scalar.activation(t0[:], u[:], ACTF.Square)
                        nc.vector.tensor_scalar(
                            out=t0[:], in0=t0[:], scalar1=0.044715 * CG, scalar2=CG,
                            op0=ALU.mult, op1=ALU.add)
                        nc.vector.tensor_tensor(out=t0[:], in0=t0[:], in1=u[:], op=ALU.mult)
                        nc.scalar.activation(t0[:], t0[:], ACTF.Tanh)
                        nc.vector.scalar_tensor_tensor(
                            out=gT[m][n][:], in0=t0[:], scalar=1.0,
                            in1=u[:], op0=ALU.add, op1=ALU.mult)
                xdm = []
                for m in range(NTT):
                    xm = wk.tile([128, C], bf16, tag="xd", bufs=8)
                    for n, (nlo, nw) in enumerate(((0, 512), (512, 256))):
                        ps = psA.tile([128, nw], f32, tag="big")
                        for k in range(NKC):
                            nc.tensor.matmul(
                                ps[:], gT[k][m // 4][:, 128 * (m % 4):128 * (m % 4 + 1)],
                                wmp_sb[:, k * C + nlo:k * C + nlo + nw],
                                start=(k == 0),
                                stop=(k == NKC - 1 and not mp_bias_nz))
                        if mp_bias_nz:
                            nc.tensor.matmul(
                                ps[:], ones_r[:1, :128],
                                bmp_sb[:1, l * C + nlo:l * C + nlo + nw],
                                start=False, stop=True)
                        nc.scalar.activation(xm[:, nlo:nlo + nw], ps[:], COPY)
                    xdm.append(xm)
                    if (m + 1) % mper == 0:
                        ch = m // mper
                        ar_launch(xdm, ar_m_in[l], ch)
                        ar_collect(ar_m_in[l], ar_m_out[l], ch)
                for ch in range(ar_chunks):
                    ar_land(xdm, ar_m_out[l], ch)

            # ---------------- final LN + lm_head (vocab shard) ----------------
            hfT = layernorm_T("lnf")
            for m in range(NM_HD):
                wh_sb = whl.tile([128, NKC * 128], bf16, tag="whd")
                nc.sync.dma_start(wh_sb[:], whd_d[m])
                lg = wk.tile([128, T], bf16, tag="lg", bufs=3)
                for n in range(2):
                    ps = psA.tile([128, 512], f32, tag="big")
                    for k in range(NKC):
                        nc.tensor.matmul(
                            ps[:], wh_sb[:, 128 * k:128 * (k + 1)],
                            hfT[k][n][:],
                            start=(k == 0), stop=(k == NKC - 1))
                    nc.scalar.activation(
                        lg[:, 512 * n:512 * (n + 1)], ps[:], ACTF.Identity,
                        bias=bhd_sb[:, m:m + 1])
                nc.sync.dma_start(logits_d[128 * m:128 * (m + 1), :], lg[:])

    nc.compile()
    return nc


def _prep(inputs, nl):
    """Host-side sharding/layout prep. Returns (in_maps, U, flags, vs_bounds)."""
    idx = np.asarray(inputs["idx"]).astype(np.int64)
    wte = _f32(np.asarray(inputs["wte"]))
    wpe = _f32(np.asarray(inputs["wpe"]))[:T]
    ln1_w = _f32(inputs["ln1_w"]); ln1_b = _f32(inputs["ln1_b"])
    ln2_w = _f32(inputs["ln2_w"]); ln2_b = _f32(inputs["ln2_b"])
    lnf_w = _f32(inputs["lnf_w"]); lnf_b = _f32(inputs["lnf_b"])
    w_attn = _f32(inputs["w_attn"]); b_attn = _f32(inputs["b_attn"])
    w_ap = _f32(inputs["w_aproj"]); b_ap = _f32(inputs["b_aproj"])
    w_fc = _f32(inputs["w_fc"]); b_fc = _f32(inputs["b_fc"])
    w_mp = _f32(inputs["w_mproj"]); b_mp = _f32(inputs["b_mproj"])
    w_hd = _f32(inputs["w_head"])

    uq, inv = np.unique(idx, return_inverse=True)
    inv = inv.reshape(idx.shape).astype(np.int32)
    U = len(uq)
    wte_c = np.ascontiguousarray(wte[uq])

    wa = w_attn * ln1_w[:, :, None]
    ba = b_attn + np.einsum("lc,lcd->ld", ln1_b, w_attn)
    wf = w_fc * ln2_w[:, :, None]
    bf = b_fc + np.einsum("lc,lcd->ld", ln2_b, w_fc)
    wh = w_hd * lnf_w[:, None]
    bh = lnf_b @ w_hd

    flags = (bool(np.any(ba[:, 2 * C:])), bool(np.any(b_ap)), bool(np.any(b_mp)))

    vs = V // TP
    rem = V - vs * TP
    bounds = []
    s0 = 0
    for g_ in range(TP):
        w_ = vs + (1 if g_ < rem else 0)
        bounds.append((s0, s0 + w_))
        s0 += w_

    def tile_lhsT(A, nm):  # A [l?, K, M] -> [l?, 128, nm*nk*128]
        l_, K, M = A.shape
        nk = K // 128
        return np.ascontiguousarray(
            A.reshape(l_, nk, 128, nm, 128).transpose(0, 2, 3, 1, 4)
            .reshape(l_, 128, nm * nk * 128))

    def tile_rhs(A):  # A [l, K, N] -> [l, 128, nk*N]
        l_, K, N = A.shape
        nk = K // 128
        return np.ascontiguousarray(
            A.reshape(l_, nk, 128, N).transpose(0, 2, 1, 3).reshape(l_, 128, nk * N))

    maps_g = []
    for g_ in range(TP):
        hs = slice(HC * g_, HC * (g_ + 1))
        q_w = wa[:nl, :, hs]
        k_w = wa[:nl, :, C + HC * g_:C + HC * (g_ + 1)]
        q_b = ba[:nl, hs]
        k_b = ba[:nl, C + HC * g_:C + HC * (g_ + 1)]
        # m-tile blocks: [q0q1 (128) | k0k1 (128) | q2 (64) | k2 (64)]
        qk_blocks = [q_w[:, :, :128], k_w[:, :, :128], q_w[:, :, 128:], k_w[:, :, 128:]]
        qk_bias_blocks = [q_b[:, :128], k_b[:, :128], q_b[:, 128:], k_b[:, 128:]]
        wv = wa[:nl, :, 2 * C + HC * g_:2 * C + HC * (g_ + 1)]
        bv = ba[:nl, 2 * C + HC * g_:2 * C + HC * (g_ + 1)]
        wap = w_ap[:nl, hs, :]
        wfc = wf[:nl, :, FS * g_:FS * (g_ + 1)]
        bfc_g = bf[:nl, FS * g_:FS * (g_ + 1)]
        wmp = w_mp[:nl, FS * g_:FS * (g_ + 1), :] * 0.5
        v0, v1 = bounds[g_]
        nv = min(v1 - v0, VPAD)  # < full only for GPT2_NMHD timing probes
        whg = np.zeros((C, VPAD), np.float32)
        whg[:, :nv] = wh[:, v0:v0 + nv]
        bhg = np.zeros((VPAD,), np.float32)
        bhg[:nv] = bh[v0:v0 + nv]

        wap_t = np.ascontiguousarray(
            wap.reshape(nl, 3, 64, C).transpose(0, 2, 1, 3).reshape(nl, 64, 3 * C))

        wqk_t = np.concatenate([tile_rhs(blk.transpose(0, 1, 2)) for blk in qk_blocks], -1)
        bqk_t = np.zeros((128, nl, 4), np.float32)
        for mi, bb in enumerate(qk_bias_blocks):
            bqk_t[:bb.shape[1], :, mi] = bb.T
        m = dict(
            wqk=_bf(wqk_t),
            wv=_bf(tile_rhs(wv)),
            wap=_bf(wap_t),
            wfc=_bf(tile_lhsT(wfc, NKC)),
            wmp=_bf(tile_rhs(wmp)),
            whd=_bf(np.ascontiguousarray(
                whg.reshape(NKC, 128, NM_HD, 128).transpose(2, 1, 0, 3)
                .reshape(NM_HD, 128, NKC * 128))),
            bqk=_f32(bqk_t.reshape(128, nl * 4)),
            bfc=_f32(bfc_g.reshape(nl, NKC, 128).transpose(2, 0, 1).reshape(128, nl * NKC)),
            bhd=_f32(bhg.reshape(NM_HD, 128).T),
            bv=_bf(bv.reshape(1, nl * HC)),
            bap=_bf((b_ap[:nl] / TP).reshape(1, nl * C)),
            bmp=_bf((b_mp[:nl] / TP).reshape(1, nl * C)),
        )
        maps_g.append(m)

    in_maps = []
    for c in range(8):
        b_, g_ = c // TP, c % TP
        m = dict(maps_g[g_])
        m["idx_t"] = np.ascontiguousarray(inv[b_].reshape(NTT, 128).T)
        m["wte_c"] = wte_c
        m["wpe_t"] = wpe
        in_maps.append(m)
    return in_maps, U, flags, bounds


def kernel(**inputs) -> np.ndarray:
    import os
    nl = int(os.environ.get("GPT2_NL", L))
    in_maps, U, flags, bounds = _prep(inputs, nl)
    key = (U, nl, flags)
    if key not in _CACHE:
        _CACHE[key] = _build(U, nl, *flags)
    nc = _CACHE[key]
    res = run_bass_kernel_spmd(nc, in_maps, core_ids=list(range(8))).results
    out = np.empty((B, T, V), np.float32)
    for c in range(8):
        b_, g_ = c // TP, c % TP
        v0, v1 = bounds[g_]
        out[b_, :, v0:v1] = res[c]["logitsT"][:v1 - v0].astype(np.float32).T
    return out


if __name__ == "__main__":
    import reference
    inputs = {k: np.asarray(v) for k, v in reference.setup_inputs().items()}
    got = kernel(**inputs)
    exp = np.asarray(reference.reference(**inputs))
    err = np.abs(got - exp).max() / np.abs(exp).max()
    denom = np.linalg.norm(exp.ravel())
    rel = np.linalg.norm((got - exp).ravel()) / denom
    print(f"absmax-rel: {err:.3e}  l2-rel: {rel:.3e}")


# revision 15
# speedup vs baseline: 3.2476x; 3.2476x over previous
"""GPT-2 (12L, 768C, 12H, T=1024, B=2) forward pass on 8 Trainium2 NeuronCores.

Sharding: 2x4 grid. Core c -> batch b=c//4, tensor-parallel shard g=c%4.
Within a batch group of 4 cores (Megatron-style):
  - attention: 3 heads per core (qkv column-sharded), output proj row-sharded,
    partial sums all-reduced (bf16) across the group.
  - MLP: fc column-sharded (768 cols/core), mproj row-sharded, all-reduced.
  - lm_head: vocab-sharded; host reassembles [B, T, V].
Residual stream x stays fp32 in SBUF in [t, c] layout; all matmuls run in
bf16 with fp32 PSUM accumulation. LayerNorm scale/bias are folded into the
following matmul's weights on the host; the device applies only
(x - mean) * rsqrt(var + eps) with torch-style unbiased variance.
rsqrt runs on the vector engine (magic-constant seed + 2 Newton steps) so the
scalar engine only ever needs {Copy, Exp, Square, Tanh} - one activation
table set, zero mid-kernel table reloads.
Attention uses a transposed-score layout (scores^T [s, t]); the softmax
denominator comes free as a 65th output row of the att@V matmul (V carries a
ones column); causal structure skips the strictly-lower-triangular compute
and masks the diagonal block.
Elementwise work is spread across engines: DVE (LN stats/normalize, softmax
divide, gelu polynomial), Activation (PSUM->SBUF copies, exp, tanh, square),
Pool (residual adds, V copies, gelu PSUM reads, collectives).
"""

import sys

sys.path.insert(0, "/opt/trn_rl_repo")

import numpy as np
import ml_dtypes

import concourse.bass as bass
import concourse.mybir as mybir
import concourse.tile as tile
from concourse import bacc
from concourse.bass_utils import run_bass_kernel_spmd
from concourse.masks import make_identity

L, H, C, V, BLK = 12, 12, 768, 50257, 1024
B, T, D = 2, 1024, 64
TP = 4                      # tensor-parallel group size
HL = H // TP                # heads per core (3)
HC = HL * D                 # head cols per core (192)
FS = 4 * C // TP            # fc cols per core (768)
NKC = C // 128              # k-tiles over C (6)
NTT = T // 128              # t-tiles (8)
import os as _os_env
NM_HD = int(_os_env.environ.get("GPT2_NMHD", "99"))  # lm-head m-tiles (99*128 = 12672 >= ceil(V/4))
VPAD = NM_HD * 128
CG = float(np.sqrt(2.0 / np.pi))
MAGIC = 0x5F3759DF
GROUPS = [[0, 1, 2, 3], [4, 5, 6, 7]]

f32 = mybir.dt.float32
bf16 = mybir.dt.bfloat16
i32 = mybir.dt.int32
ALU = mybir.AluOpType
ACTF = mybir.ActivationFunctionType
COPY = ACTF.Copy

_CACHE = {}


def _bf(a):
    return np.ascontiguousarray(a.astype(ml_dtypes.bfloat16))


def _f32(a):
    return np.ascontiguousarray(a.astype(np.float32))


def _build(U, nl, v_bias_nz, ap_bias_nz, mp_bias_nz):
    nc = bacc.Bacc("TRN2", target_bir_lowering=False, debug=False, num_devices=8)

    # ---------------- DRAM declarations ----------------
    idx_d = nc.dram_tensor("idx_t", [128, NTT], i32, kind="ExternalInput")
    wte_d = nc.dram_tensor("wte_c", [U, C], f32, kind="ExternalInput")
    wpe_d = nc.dram_tensor("wpe_t", [T, C], f32, kind="ExternalInput")
    wqk_d = nc.dram_tensor("wqk", [nl, 128, 3 * NKC * 128], bf16, kind="ExternalInput")
    wv_d = nc.dram_tensor("wv", [nl, 128, NKC * HC], bf16, kind="ExternalInput")
    wap_d = nc.dram_tensor("wap", [nl, 64, 3 * C], bf16, kind="ExternalInput")
    wfc_d = nc.dram_tensor("wfc", [nl, 128, NKC * NKC * 128], bf16, kind="ExternalInput")
    wmp_d = nc.dram_tensor("wmp", [nl, 128, NKC * C], bf16, kind="ExternalInput")
    whd_d = nc.dram_tensor("whd", [NM_HD, 128, NKC * 128], bf16, kind="ExternalInput")
    bqk_d = nc.dram_tensor("bqk", [128, nl * 4], f32, kind="ExternalInput")
    bfc_d = nc.dram_tensor("bfc", [128, nl * NKC], f32, kind="ExternalInput")
    bhd_d = nc.dram_tensor("bhd", [128, NM_HD], f32, kind="ExternalInput")
    bv_d = nc.dram_tensor("bv", [1, nl * HC], bf16, kind="ExternalInput")
    bap_d = nc.dram_tensor("bap", [1, nl * C], bf16, kind="ExternalInput")
    bmp_d = nc.dram_tensor("bmp", [1, nl * C], bf16, kind="ExternalInput")
    logits_d = nc.dram_tensor("logitsT", [VPAD, T], bf16, kind="ExternalOutput")

    ar_a_in = [nc.dram_tensor(f"ar_a_in{l}", [T, C], bf16) for l in range(nl)]
    ar_a_out = [nc.dram_tensor(f"ar_a_out{l}", [T, C], bf16) for l in range(nl)]
    ar_m_in = [nc.dram_tensor(f"ar_m_in{l}", [T, C], bf16) for l in range(nl)]
    ar_m_out = [nc.dram_tensor(f"ar_m_out{l}", [T, C], bf16) for l in range(nl)]

    with tile.TileContext(nc) as tc:
        with (
            tc.tile_pool(name="const", bufs=1) as cpool,
            tc.tile_pool(name="xres", bufs=1) as xpool,
            tc.tile_pool(name="work", bufs=2) as wk,
            tc.tile_pool(name="hT", bufs=13) as hTp,
            tc.tile_pool(name="qkp", bufs=4) as qkp,
            tc.tile_pool(name="vtp", bufs=9) as vtp,
            tc.tile_pool(name="oTp", bufs=4) as oTp,
            tc.tile_pool(name="expT", bufs=9) as expp,
            tc.tile_pool(name="wload", bufs=2) as wl,
            tc.tile_pool(name="whload", bufs=3) as whl,
            tc.tile_pool(name="psA", bufs=3, space="PSUM") as psA,
            tc.tile_pool(name="psT", bufs=2, space="PSUM") as psT,
            tc.tile_pool(name="psS", bufs=2, space="PSUM") as psS,
        ):
            # ---------------- constants ----------------
            ident = cpool.tile([128, 128], bf16)
            make_identity(nc, ident[:])
            tril = cpool.tile([128, 128], bf16)   # keep s<=t  (p<=f)
            nc.gpsimd.memset(tril[:], 1.0)
            nc.gpsimd.affine_select(
                out=tril[:], in_=tril[:], compare_op=ALU.is_ge, fill=0.0,
                base=0, pattern=[[1, 128]], channel_multiplier=-1)
            ones_s = cpool.tile([128, 64], bf16)
            nc.gpsimd.memset(ones_s[:], 1.0)
            ones_r = cpool.tile([1, 128], bf16)
            nc.gpsimd.memset(ones_r[:], 1.0)
            bqk_sb = cpool.tile([128, nl * 4], f32)
            nc.sync.dma_start(bqk_sb[:], bqk_d[:])
            bfc_sb = cpool.tile([128, nl * NKC], f32)
            nc.sync.dma_start(bfc_sb[:], bfc_d[:])
            bhd_sb = cpool.tile([128, NM_HD], f32)
            nc.sync.dma_start(bhd_sb[:], bhd_d[:])
            bv_sb = bap_sb = bmp_sb = None
            if v_bias_nz:
                bv_sb = cpool.tile([1, nl * HC], bf16)
                nc.sync.dma_start(bv_sb[:], bv_d[:])
            if ap_bias_nz:
                bap_sb = cpool.tile([1, nl * C], bf16)
                nc.sync.dma_start(bap_sb[:], bap_d[:])
            if mp_bias_nz:
                bmp_sb = cpool.tile([1, nl * C], bf16)
                nc.sync.dma_start(bmp_sb[:], bmp_d[:])

            # ---------------- embedding ----------------
            no_emb = _os_env.environ.get("GPT2_NO_EMB", "0") == "1"  # timing probe
            idx_sb = cpool.tile([128, NTT], i32)
            nc.sync.dma_start(idx_sb[:], idx_d[:])
            x = []
            for j in range(NTT):
                p_t = wk.tile([128, C], f32, tag="pos")
                nc.sync.dma_start(p_t[:], wpe_d[128 * j:128 * (j + 1), :])
                xt = xpool.tile([128, C], f32, tag=f"x{j}")
                if no_emb:
                    nc.vector.tensor_copy(xt[:], p_t[:])
                else:
                    g_t = wk.tile([128, C], f32, tag="emb")
                    nc.gpsimd.indirect_dma_start(
                        out=g_t[:], out_offset=None, in_=wte_d[:],
                        in_offset=bass.IndirectOffsetOnAxis(ap=idx_sb[:, j:j + 1], axis=0))
                    nc.gpsimd.tensor_tensor(out=xt[:], in0=g_t[:], in1=p_t[:], op=ALU.add)
                x.append(xt)

            # ---------------- helpers ----------------
            def layernorm_T(tag):
                """LN(x) (no affine; folded into weights), bf16, transposed.
                Returns 6 tiles [128, T] = h^T (c on partitions).
                rsqrt on DVE (magic seed + 2 Newton steps): the scalar engine
                never sees Sqrt, so its activation table is never reloaded."""
                hT = [[hTp.tile([128, 512], bf16, tag="hT", name=f"hT{_k}_{_n}")
                       for _n in range(2)] for _k in range(NKC)]
                stats = wk.tile([128, NTT, 2], f32, tag="lnstats")
                for j in range(NTT):
                    st6 = wk.tile([128, 2, 6], f32, tag="st6")
                    nc.vector.bn_stats(st6[:, 0, :], x[j][:, 0:384])
                    nc.vector.bn_stats(st6[:, 1, :], x[j][:, 384:768])
                    nc.vector.bn_aggr(stats[:, j, :], st6[:])
                # istd = rsqrt(var * C/(C-1) + eps) for all 8 tiles at once
                vc = wk.tile([128, NTT], f32, tag="ln_vc")
                nc.vector.tensor_scalar(
                    out=vc[:], in0=stats[:, :, 1], scalar1=float(C / (C - 1)),
                    scalar2=1e-5, op0=ALU.mult, op1=ALU.add)
                ti = wk.tile([128, NTT], i32, tag="ln_ti")
                nc.vector.tensor_scalar(
                    out=ti[:], in0=vc[:].bitcast(i32), scalar1=1, scalar2=None,
                    op0=ALU.arith_shift_right)
                nc.vector.tensor_scalar(
                    out=ti[:], in0=ti[:], scalar1=-1, scalar2=None,
                    op0=ALU.bitwise_xor)
                yi = wk.tile([128, NTT], i32, tag="ln_yi")
                nc.vector.tensor_scalar(
                    out=yi[:], in0=ti[:], scalar1=MAGIC + 1, scalar2=None,
                    op0=ALU.add)
                y = yi[:].bitcast(f32)
                yy = wk.tile([128, NTT], f32, tag="ln_yy")
                for _ in range(2):  # Newton: y *= 1.5 - 0.5*v*y*y
                    nc.vector.tensor_tensor(out=yy[:], in0=y, in1=y, op=ALU.mult)
                    nc.vector.tensor_tensor(out=yy[:], in0=yy[:], in1=vc[:], op=ALU.mult)
                    nc.vector.tensor_scalar(
                        out=yy[:], in0=yy[:], scalar1=-0.5, scalar2=1.5,
                        op0=ALU.mult, op1=ALU.add)
                    nc.vector.tensor_tensor(out=y, in0=y, in1=yy[:], op=ALU.mult)
                nmean = wk.tile([128, NTT], f32, tag="ln_nm")
                nc.vector.tensor_scalar_mul(nmean[:], stats[:, :, 0], -1.0)
                for j in range(NTT):
                    hb = wk.tile([128, C], bf16, tag="hbf")
                    nc.vector.tensor_scalar(
                        out=hb[:], in0=x[j][:], scalar1=nmean[:, j:j + 1],
                        scalar2=y[:, j:j + 1], op0=ALU.add, op1=ALU.mult)
                    for k in range(NKC):
                        ps = psT.tile([128, 128], bf16, tag="tp")
                        nc.tensor.transpose(ps[:], hb[:, 128 * k:128 * (k + 1)], ident[:])
                        nc.scalar.activation(
                            hT[k][j // 4][:, 128 * (j % 4):128 * (j % 4 + 1)],
                            ps[:], COPY)
                return hT

            import os as _os
            no_ar = _os.environ.get("GPT2_NO_AR", "0") == "1"
            ar_chunks = int(_os.environ.get("GPT2_AR_CHUNKS", "2"))
            mper = NTT // ar_chunks

            def ar_launch(xd_tiles, cc_in, ch):
                """DMA chunk ch of the partial delta out and start its AR."""
                if no_ar:
                    return
                for m in range(ch * mper, (ch + 1) * mper):
                    nc.sync.dma_start(cc_in[128 * m:128 * (m + 1), :], xd_tiles[m][:])

            def ar_collect(cc_in, cc_out, ch):
                if no_ar:
                    return
                r0, r1 = 128 * ch * mper, 128 * (ch + 1) * mper
                nc.gpsimd.collective_compute(
                    "AllReduce", ALU.add, replica_groups=GROUPS,
                    ins=[cc_in[r0:r1, :].opt()], outs=[cc_out[r0:r1, :].opt()])

            def ar_land(xd_tiles, cc_out, ch):
                """DMA chunk ch of the reduced delta back and add into x."""
                for m in range(ch * mper, (ch + 1) * mper):
                    if no_ar:
                        nc.gpsimd.tensor_tensor(
                            out=x[m][:], in0=x[m][:], in1=xd_tiles[m][:], op=ALU.add)
                        continue
                    ard = wk.tile([128, C], bf16, tag="ard")
                    nc.gpsimd.dma_start(ard[:], cc_out[128 * m:128 * (m + 1), :])
                    nc.gpsimd.tensor_tensor(
                        out=x[m][:], in0=x[m][:], in1=ard[:], op=ALU.add)

            # ---------------- transformer layers ----------------
            for l in range(nl):
                # layer weight loads (double-buffered; overlap with compute)
                wqk_sb = wl.tile([128, 3 * NKC * 128], bf16, tag="wqk")
                nc.sync.dma_start(wqk_sb[:], wqk_d[l])
                wv_sb = wl.tile([128, NKC * HC], bf16, tag="wv")
                nc.sync.dma_start(wv_sb[:], wv_d[l])
                wap_sb = wl.tile([64, 3 * C], bf16, tag="wap")
                nc.sync.dma_start(wap_sb[:], wap_d[l])
                wfc_sb = wl.tile([128, NKC * NKC * 128], bf16, tag="wfc", bufs=1)
                nc.sync.dma_start(wfc_sb[:], wfc_d[l])
                wmp_sb = wl.tile([128, NKC * C], bf16, tag="wmp", bufs=1)
                nc.sync.dma_start(wmp_sb[:], wmp_d[l])

                # ---- ln1 + qkv ----
                hT = layernorm_T(f"ln1_{l}")
                qk_meta = ((128, 0), (128, 768), (64, 1536), (64, 1920))
                qkT = [qkp.tile([mw, T], bf16, tag="qkT", name=f"qk{m}")
                       for m, (mw, moff) in enumerate(qk_meta)]
                for n in range(2):
                    for m, (mw, moff) in enumerate(qk_meta):
                        ps = psA.tile([mw, 512], f32, tag="big")
                        for k in range(NKC):
                            nc.tensor.matmul(
                                ps[:], wqk_sb[:, moff + k * mw:moff + (k + 1) * mw],
                                hT[k][n][:],
                                start=(k == 0), stop=(k == NKC - 1))
                        nc.scalar.activation(
                            qkT[m][:, 512 * n:512 * (n + 1)], ps[:], ACTF.Identity,
                            bias=bqk_sb[:mw, l * 4 + m:l * 4 + m + 1])

                vt = []
                for m in range(NTT):
                    vm = vtp.tile([128, HL, 65], bf16, tag="vt")
                    ps = psA.tile([128, HC], f32, tag="big")
                    nk = NKC + (1 if v_bias_nz else 0)
                    for k in range(NKC):
                        nc.tensor.matmul(
                            ps[:], hT[k][m // 4][:, 128 * (m % 4):128 * (m % 4 + 1)],
                            wv_sb[:, k * HC:(k + 1) * HC],
                            start=(k == 0), stop=(k == nk - 1))
                    if v_bias_nz:
                        nc.tensor.matmul(
                            ps[:], ones_r[:1, :128],
                            bv_sb[:1, l * HC:(l + 1) * HC],
                            start=False, stop=True)
                    for hh in range(HL):
                        nc.vector.tensor_copy(
                            vm[:, hh, 0:64], ps[:, 64 * hh:64 * (hh + 1)])
                    nc.gpsimd.memset(vm[:, :, 64:65], 1.0)
                    vt.append(vm)

                # ---- attention (scores^T layout, causal skip, free denom) ----
                oT = [oTp.tile([64, T], bf16, tag="oT", name=f"oT{_h}")
                      for _h in range(HL)]
                # ex[h][i]: exp'd scores^T for s-block i; only t >= 128*i kept
                ex = [[expp.tile([128, T - 128 * _i], bf16, tag=f"expT{_i}",
                                 name=f"ex{_h}_{_i}", bufs=3)
                       for _i in range(NTT)] for _h in range(HL)]
                xda = []
                for cch in range(2):
                    nblk = min(NTT, 4 * (cch + 1))
                    for hh in range(HL):
                        if hh < 2:
                            qT = qkT[0][64 * hh:64 * hh + 64, :]
                            kT = qkT[1][64 * hh:64 * hh + 64, :]
                        else:
                            qT = qkT[2][0:64, :]
                            kT = qkT[3][0:64, :]
                        for i in range(nblk):
                            toff = 128 * i
                            lo = max(toff, 512 * cch)
                            hi = 512 * (cch + 1)
                            if lo >= hi:
                                continue
                            ps = psA.tile([128, hi - lo], f32, tag="big")
                            nc.tensor.matmul(
                                ps[:], kT[:, toff:toff + 128],
                                qT[:, lo:hi], start=True, stop=True)
                            nc.scalar.activation(
                                ex[hh][i][:, lo - toff:hi - toff], ps[:], ACTF.Exp,
                                scale=float(D ** -0.5))
                            if lo == toff:  # diagonal block: mask s>t
                                nc.vector.tensor_tensor(
                                    out=ex[hh][i][:, 0:128],
                                    in0=ex[hh][i][:, 0:128], in1=tril[:],
                                    op=ALU.mult)
                        po = psS.tile([65, 512], f32, tag="po")
                        for i in range(nblk):
                            toff = 128 * i
                            lo = max(toff - 512 * cch, 0)
                            nc.tensor.matmul(
                                po[:, lo:512], vt[i][:, hh, :],
                                ex[hh][i][:, 512 * cch + lo - toff:
                                          512 * (cch + 1) - toff],
                                start=(i == 0), stop=(i == nblk - 1))
                        den = wk.tile([128, 512], bf16, tag="den")
                        nc.scalar.activation(den[64:65, :], po[64:65, :], COPY)
                        pb = psS.tile([64, 512], f32, tag="pb", bufs=1)
                        nc.tensor.matmul(
                            pb[:], ones_s[64:65, :], den[64:65, :],
                            start=True, stop=True)
                        rb = wk.tile([64, 512], f32, tag="rb")
                        nc.vector.reciprocal(rb[:], pb[:])
                        nc.vector.tensor_tensor(
                            out=oT[hh][:, 512 * cch:512 * (cch + 1)],
                            in0=po[0:64, :], in1=rb[:], op=ALU.mult)

                    # ---- attention output projection for this T-half ----
                    for m in range(4 * cch, 4 * (cch + 1)):
                        xm = wk.tile([128, C], bf16, tag="xd", bufs=8)
                        for n, (nlo, nw) in enumerate(((0, 512), (512, 256))):
                            ps = psA.tile([128, nw], f32, tag="big")
                            for kk in range(HL):
                                nc.tensor.matmul(
                                    ps[:], oT[kk][:, 128 * m:128 * (m + 1)],
                                    wap_sb[:, kk * C + nlo:kk * C + nlo + nw],
                                    start=(kk == 0),
                                    stop=(kk == HL - 1 and not ap_bias_nz))
                            if ap_bias_nz:
                                nc.tensor.matmul(
                                    ps[:], ones_r[:1, :128],
                                    bap_sb[:1, l * C + nlo:l * C + nlo + nw],
                                    start=False, stop=True)
                            nc.scalar.activation(xm[:, nlo:nlo + nw], ps[:], COPY)
                        xda.append(xm)
                    ch0 = 4 * cch // mper
                    for ch in range(ch0, (4 * (cch + 1)) // mper):
                        ar_launch(xda, ar_a_in[l], ch)
                        ar_collect(ar_a_in[l], ar_a_out[l], ch)
                for ch in range(ar_chunks):
                    ar_land(xda, ar_a_out[l], ch)

                # ---- ln2 + MLP ----
                h2T = layernorm_T(f"ln2_{l}")
                gT = [[hTp.tile([128, 512], bf16, tag="gT", name=f"gT{_k}_{_n}")
                      for _n in range(2)] for _k in range(NKC)]
                for n in range(2):
                    for m in range(NKC):
                        ps = psA.tile([128, 512], f32, tag="big")
                        for k in range(NKC):
                            nc.tensor.matmul(
                                ps[:], wfc_sb[:, (m * NKC + k) * 128:(m * NKC + k + 1) * 128],
                                h2T[k][n][:],
                                start=(k == 0), stop=(k == NKC - 1))
                        # u = ps + b ; gelu*2 = u * (1 + tanh(cg*(u + 0.044715 u^3)))
                        u = wk.tile([128, 512], f32, tag="gelu_u")
                        nc.vector.tensor_scalar_add(
                            u[:], ps[:], bfc_sb[:, l * NKC + m:l * NKC + m + 1])
                        t0 = wk.tile([128, 512], f32, tag="gelu_t0")
                        nc.scalar.activation(t0[:], u[:], ACTF.Square)
                        nc.vector.tensor_scalar(
                            out=t0[:], in0=t0[:], scalar1=0.044715 * CG, scalar2=CG,
                            op0=ALU.mult, op1=ALU.add)
                        nc.vector.tensor_tensor(out=t0[:], in0=t0[:], in1=u[:], op=ALU.mult)
                        nc.scalar.activation(t0[:], t0[:], ACTF.Tanh)
                        nc.vector.scalar_tensor_tensor(
                            out=gT[m][n][:], in0=t0[:], scalar=1.0,
                            in1=u[:], op0=ALU.add, op1=ALU.mult)
                xdm = []
                for m in range(NTT):
                    xm = wk.tile([128, C], bf16, tag="xd", bufs=8)
                    for n, (nlo, nw) in enumerate(((0, 512), (512, 256))):
                        ps = psA.tile([128, nw], f32, tag="big")
                        for k in range(NKC):
                            nc.tensor.matmul(
                                ps[:], gT[k][m // 4][:, 128 * (m % 4):128 * (m % 4 + 1)],
                                wmp_sb[:, k * C + nlo:k * C + nlo + nw],
                                start=(k == 0),
                                stop=(k == NKC - 1 and not mp_bias_nz))
                        if mp_bias_nz:
                            nc.tensor.matmul(
                                ps[:], ones_r[:1, :128],
                                bmp_sb[:1, l * C + nlo:l * C + nlo + nw],
                                start=False, stop=True)
                        nc.scalar.activation(xm[:, nlo:nlo + nw], ps[:], COPY)
                    xdm.append(xm)
                    if (m + 1) % mper == 0:
                        ch = m // mper
                        ar_launch(xdm, ar_m_in[l], ch)
                        ar_collect(ar_m_in[l], ar_m_out[l], ch)
                for ch in range(ar_chunks):
                    ar_land(xdm, ar_m_out[l], ch)

            # ---------------- final LN + lm_head (vocab shard) ----------------
            hfT = layernorm_T("lnf")
            for m in range(NM_HD):
                wh_sb = whl.tile([128, NKC * 128], bf16, tag="whd")
                nc.sync.dma_start(wh_sb[:], whd_d[m])
                lg = wk.tile([128, T], bf16, tag="lg", bufs=3)
                for n in range(2):
                    ps = psA.tile([128, 512], f32, tag="big")
                    for k in range(NKC):
                        nc.tensor.matmul(
                            ps[:], wh_sb[:, 128 * k:128 * (k + 1)],
                            hfT[k][n][:],
                            start=(k == 0), stop=(k == NKC - 1))
                    nc.scalar.activation(
                        lg[:, 512 * n:512 * (n + 1)], ps[:], ACTF.Identity,
                        bias=bhd_sb[:, m:m + 1])
                nc.sync.dma_start(logits_d[128 * m:128 * (m + 1), :], lg[:])

    nc.compile()
    return nc


def _prep(inputs, nl):
    """Host-side sharding/layout prep. Returns (in_maps, U, flags, vs_bounds)."""
    idx = np.asarray(inputs["idx"]).astype(np.int64)
    wte = _f32(np.asarray(inputs["wte"]))
    wpe = _f32(np.asarray(inputs["wpe"]))[:T]
    ln1_w = _f32(inputs["ln1_w"]); ln1_b = _f32(inputs["ln1_b"])
    ln2_w = _f32(inputs["ln2_w"]); ln2_b = _f32(inputs["ln2_b"])
    lnf_w = _f32(inputs["lnf_w"]); lnf_b = _f32(inputs["lnf_b"])
    w_attn = _f32(inputs["w_attn"]); b_attn = _f32(inputs["b_attn"])
    w_ap = _f32(inputs["w_aproj"]); b_ap = _f32(inputs["b_aproj"])
    w_fc = _f32(inputs["w_fc"]); b_fc = _f32(inputs["b_fc"])
    w_mp = _f32(inputs["w_mproj"]); b_mp = _f32(inputs["b_mproj"])
    w_hd = _f32(inputs["w_head"])

    uq, inv = np.unique(idx, return_inverse=True)
    inv = inv.reshape(idx.shape).astype(np.int32)
    U = len(uq)
    wte_c = np.ascontiguousarray(wte[uq])

    wa = w_attn * ln1_w[:, :, None]
    ba = b_attn + np.einsum("lc,lcd->ld", ln1_b, w_attn)
    wf = w_fc * ln2_w[:, :, None]
    bf = b_fc + np.einsum("lc,lcd->ld", ln2_b, w_fc)
    wh = w_hd * lnf_w[:, None]
    bh = lnf_b @ w_hd

    flags = (bool(np.any(ba[:, 2 * C:])), bool(np.any(b_ap)), bool(np.any(b_mp)))

    vs = V // TP
    rem = V - vs * TP
    bounds = []
    s0 = 0
    for g_ in range(TP):
        w_ = vs + (1 if g_ < rem else 0)
        bounds.append((s0, s0 + w_))
        s0 += w_

    def tile_lhsT(A, nm):  # A [l?, K, M] -> [l?, 128, nm*nk*128]
        l_, K, M = A.shape
        nk = K // 128
        return np.ascontiguousarray(
            A.reshape(l_, nk, 128, nm, 128).transpose(0, 2, 3, 1, 4)
            .reshape(l_, 128, nm * nk * 128))

    def tile_rhs(A):  # A [l, K, N] -> [l, 128, nk*N]
        l_, K, N = A.shape
        nk = K // 128
        return np.ascontiguousarray(
            A.reshape(l_, nk, 128, N).transpose(0, 2, 1, 3).reshape(l_, 128, nk * N))

    maps_g = []
    for g_ in range(TP):
        hs = slice(HC * g_, HC * (g_ + 1))
        q_w = wa[:nl, :, hs]
        k_w = wa[:nl, :, C + HC * g_:C + HC * (g_ + 1)]
        q_b = ba[:nl, hs]
        k_b = ba[:nl, C + HC * g_:C + HC * (g_ + 1)]
        # m-tile blocks: [q0q1 (128) | k0k1 (128) | q2 (64) | k2 (64)]
        qk_blocks = [q_w[:, :, :128], k_w[:, :, :128], q_w[:, :, 128:], k_w[:, :, 128:]]
        qk_bias_blocks = [q_b[:, :128], k_b[:, :128], q_b[:, 128:], k_b[:, 128:]]
        wv = wa[:nl, :, 2 * C + HC * g_:2 * C + HC * (g_ + 1)]
        bv = ba[:nl, 2 * C + HC * g_:2 * C + HC * (g_ + 1)]
        wap = w_ap[:nl, hs, :]
        wfc = wf[:nl, :, FS * g_:FS * (g_ + 1)]
        bfc_g = bf[:nl, FS * g_:FS * (g_ + 1)]
        wmp = w_mp[:nl, FS * g_:FS * (g_ + 1), :] * 0.5
        v0, v1 = bounds[g_]
        nv = min(v1 - v0, VPAD)  # < full only for GPT2_NMHD timing probes
        whg = np.zeros((C, VPAD), np.float32)
        whg[:, :nv] = wh[:, v0:v0 + nv]
        bhg = np.zeros((VPAD,), np.float32)
        bhg[:nv] = bh[v0:v0 + nv]

        wap_t = np.ascontiguousarray(
            wap.reshape(nl, 3, 64, C).transpose(0, 2, 1, 3).reshape(nl, 64, 3 * C))

        wqk_t = np.concatenate([tile_rhs(blk.transpose(0, 1, 2)) for blk in qk_blocks], -1)
        bqk_t = np.zeros((128, nl, 4), np.float32)
        for mi, bb in enumerate(qk_bias_blocks):
            bqk_t[:bb.shape[1], :, mi] = bb.T
        m = dict(
            wqk=_bf(wqk_t),
            wv=_bf(tile_rhs(wv)),
            wap=_bf(wap_t),
            wfc=_bf(tile_lhsT(wfc, NKC)),
            wmp=_bf(tile_rhs(wmp)),
            whd=_bf(np.ascontiguousarray(
                whg.reshape(NKC, 128, NM_HD, 128).transpose(2, 1, 0, 3)
                .reshape(NM_HD, 128, NKC * 128))),
            bqk=_f32(bqk_t.reshape(128, nl * 4)),
            bfc=_f32(bfc_g.reshape(nl, NKC, 128).transpose(2, 0, 1).reshape(128, nl * NKC)),
            bhd=_f32(bhg.reshape(NM_HD, 128).T),
            bv=_bf(bv.reshape(1, nl * HC)),
            bap=_bf((b_ap[:nl] / TP).reshape(1, nl * C)),
            bmp=_bf((b_mp[:nl] / TP).reshape(1, nl * C)),
        )
        maps_g.append(m)

    in_maps = []
    for c in range(8):
        b_, g_ = c // TP, c % TP
        m = dict(maps_g[g_])
        m["idx_t"] = np.ascontiguousarray(inv[b_].reshape(NTT, 128).T)
        m["wte_c"] = wte_c
        m["wpe_t"] = wpe
        in_maps.append(m)
    return in_maps, U, flags, bounds


def kernel(**inputs) -> np.ndarray:
    import os
    nl = int(os.environ.get("GPT2_NL", L))
    in_maps, U, flags, bounds = _prep(inputs, nl)
    key = (U, nl, flags)
    if key not in _CACHE:
        _CACHE[key] = _build(U, nl, *flags)
    nc = _CACHE[key]
    res = run_bass_kernel_spmd(nc, in_maps, core_ids=list(range(8))).results
    out = np.empty((B, T, V), np.float32)
    for c in range(8):
        b_, g_ = c // TP, c % TP
        v0, v1 = bounds[g_]
        out[b_, :, v0:v1] = res[c]["logitsT"][:v1 - v0].astype(np.float32).T
    return out


if __name__ == "__main__":
    import reference
    inputs = {k: np.asarray(v) for k, v in reference.setup_inputs().items()}
    got = kernel(**inputs)
    exp = np.asarray(reference.reference(**inputs))
    err = np.abs(got - exp).max() / np.abs(exp).max()
    denom = np.linalg.norm(exp.ravel())
    rel = np.linalg.norm((got - exp).ravel()) / denom
    print(f"absmax-rel: {err:.3e}  l2-rel: {rel:.3e}")
